# revision 36
# baseline (speedup 1.0000x reference)
"""bf16 variant: 4-way PE-quadrant packing + once-per-image edge compute.

Main conv (d=4): four concurrent K=64/M=64 bf16 matmuls occupy the four 64x64
quadrants of the PE array (tile positions (0,0), (0,64), (64,0), (64,64)):
lower/upper image half on array rows, even/odd row-pair on array columns.
Each group of 4 row-pairs runs 9 taps x 4 units; evacuation (bias add,
psum->sbuf) alternates between ScalarE and VectorE.

Timeline engineering:
- PE warmup: ~16 dummy taps on a memset scratch tile run during the initial
  DMA window so the TensorE p-state is at max clock when real data lands.
- Startup-critical DMA order: main-tap weights (148KB), then strip 0 as two
  independent tiles (slots 0-9 / 8-17, 2-slot halo duplicated) so groups 0-1
  start as soon as the first half lands. Edge tensors stream behind strips
  1-2.
- Edge frame (convs d in {7,1,5,3}) is computed once per image AFTER the
  dense loop (overlapping the output-DMA drain), with left/right chains
  crossed over psum partition halves so all four PE quadrants run. The four
  corner pixels are fixed up on the host. Host overlays edges onto the dense
  result. Dense + edge outputs are bf16 (error budget allows it).
"""

import ml_dtypes
import numpy as np

import concourse.bacc as bacc
import concourse.mybir as mybir
import concourse.tile as tile
from concourse.bass import ts
from concourse.bass_utils import run_bass_kernel_spmd

B, C, H, W = 8, 64, 256, 256
NCORES = 8
R = 32              # output rows per strip
H2 = R // 2         # rows per partition-half
NSTRIP = H // R
SLOTS = H2 + 2
WPAD = W + 2
XCOLS = SLOTS * WPAD
NG = H2 // 4        # groups of 4 row-pairs per half
F32 = mybir.dt.float32
BF16 = mybir.dt.bfloat16
AF = mybir.ActivationFunctionType
BF = ml_dtypes.bfloat16
NWARM = 9           # PE p-state warmup taps (~0.43us each at mid clock);
                    # must END at/after first-input-ready: an idle Tensor gap
                    # between warmup and real work resets the p-state ramp

TAPS9 = [(dy, dx) for dy in (-1, 0, 1) for dx in (-1, 0, 1)]
TOP_TAPS = [(dy, dx) for dy in (0, 1) for dx in (-1, 0, 1)]      # d=7 row 0
BOT_TAPS = [(dy, dx) for dy in (-1, 0) for dx in (-1, 0, 1)]     # d=1 row 255
LEFT_TAPS = [(dy, dx) for dy in (-1, 0, 1) for dx in (0, 1)]     # d=5 col 0
RIGHT_TAPS = [(dy, dx) for dy in (-1, 0, 1) for dx in (-1, 0)]   # d=3 col 255

W_GROUPS = [
    (4, TAPS9), (7, TOP_TAPS), (1, BOT_TAPS), (5, LEFT_TAPS), (3, RIGHT_TAPS),
]
_offs = []
_acc = 0
for _d, _taps in W_GROUPS:
    _offs.append(_acc)
    _acc += len(_taps)
(MAIN_S, TOP_S, BOT_S, LEFT_S, RIGHT_S) = _offs
NW = _acc  # 33
NWM = len(TAPS9)       # main-tap weight columns (first group)
NWE = NW - NWM         # edge-tap weight columns
# bias tile [128, NB]: column -> (value on partitions 0-63, on 64-127)
BIAS_PAIRS = [(4, 4), (5, 5), (3, 3), (7, 1)]
B_MAIN, B_LEFT, B_RIGHT, B_TOPBOT = range(4)
NB = 4

# edge_in sections (element offsets per partition)
LCOL, RCOL, TOPS, BOTS = 0, 516, 1032, 1548
EIN = 2064
EOUT = 512  # edges_out: [0:128] left, [128:256] right (row-half crossed),
            # [256:512] top|bottom

_CACHE = {}


def _build():
    nc = bacc.Bacc("TRN2", target_bir_lowering=False, debug=False,
                   num_devices=NCORES)
    ip = nc.dram_tensor("img_prep", [NSTRIP, 128, XCOLS], BF16,
                        kind="ExternalInput").ap()
    ein_d = nc.dram_tensor("edge_in", [128, EIN], BF16,
                           kind="ExternalInput").ap()
    wtm_d = nc.dram_tensor("wtm", [128, NWM * 64], BF16,
                           kind="ExternalInput").ap()
    wte_d = nc.dram_tensor("wte", [128, NWE * 64], BF16,
                           kind="ExternalInput").ap()
    bias_d = nc.dram_tensor("bias", [128, NB], F32, kind="ExternalInput").ap()
    out_d = nc.dram_tensor("out", [NSTRIP, 2, 128, H2 * 254 // 2], BF16,
                           kind="ExternalOutput").ap()
    edg_d = nc.dram_tensor("edges", [128, EOUT], BF16,
                           kind="ExternalOutput").ap()
    warm_d = nc.dram_tensor("warm", [128, 4], F32, kind="ExternalOutput").ap()

    lo, up = slice(0, 64), slice(64, 128)
    quads = ((lo, slice(0, 64)), (up, slice(0, 64)),
             (lo, slice(64, 128)), (up, slice(64, 128)))

    with tile.TileContext(nc) as tc:
        with (
            tc.tile_pool(name="const", bufs=1) as constp,
            tc.tile_pool(name="xin", bufs=5) as xp,
            tc.tile_pool(name="outp", bufs=5) as op,
            tc.tile_pool(name="psmain", bufs=4, space="PSUM") as pp,
        ):
            # ---- PE p-state warmup on a memset scratch (no DMA deps) ----
            if NWARM:
                warm = constp.tile([128, 576], BF16)
                nc.gpsimd.memset(warm[:], 0.0)
                wps1 = pp.tile([128, 512], F32, tag="ps1")
                wps2 = pp.tile([128, 512], F32, tag="ps2")
                for k in range(NWARM):
                    st, sp = (k == 0), (k == NWARM - 1)
                    for (ph, po) in quads:
                        psd = (wps1 if ph == lo else wps2)
                        nc.tensor.matmul(psd[po, :], warm[ph, 0:64],
                                         warm[ph, 64:576], start=st, stop=sp,
                                         skip_group_check=True)

            # ---- startup-critical DMAs first on the Sync hardware ring
            # (other engines' dma_start lands on the slow Q0 software path).
            # Strip 0 comes as 3 tiles (6/10/6 slots, 2-slot halos duplicated)
            # so group 0 can start on a 0.4MB chunk while the ring ramps.
            wtm = constp.tile([128, NWM * 64], BF16)
            nc.sync.dma_start(wtm[:], wtm_d[:])
            X0a = constp.tile([128, 6 * WPAD], BF16)
            nc.sync.dma_start(X0a[:], ip[0][:, 0:6 * WPAD])
            X0b = constp.tile([128, 10 * WPAD], BF16)
            nc.sync.dma_start(X0b[:], ip[0][:, 4 * WPAD:14 * WPAD])
            X0c = constp.tile([128, 6 * WPAD], BF16)
            nc.sync.dma_start(X0c[:], ip[0][:, 12 * WPAD:18 * WPAD])
            bias_t = constp.tile([128, NB], F32)
            nc.sync.dma_start(bias_t[:], bias_d[:])
            Xs = {}
            for s in (1, 2):
                Xs[s] = xp.tile([128, XCOLS], BF16, name=f"X{s}", tag="X")
                nc.sync.dma_start(Xs[s][:], ip[s])
            # edge tensors + warmup readback stream behind the first strips
            ein = constp.tile([128, EIN], BF16)
            nc.sync.dma_start(ein[:], ein_d[:])
            wte = constp.tile([128, NWE * 64], BF16)
            nc.sync.dma_start(wte[:], wte_d[:])
            if NWARM:
                # readback gives the warmup psum tiles readers; emitted after
                # the descriptor issues so it never gates them
                wsb = constp.tile([128, 4], F32)
                nc.scalar.activation(wsb[:, 0:2], wps1[:, 0:2], AF.Identity)
                nc.scalar.activation(wsb[:, 2:4], wps2[:, 0:2], AF.Identity)
                nc.sync.dma_start(warm_d[:], wsb[:])

            # ---- edge frame emitter (called between strips 6 and 7 so the
            # edge evac/DMA overlap the last strip's compute) ----
            Elc = ein[:, LCOL:LCOL + 516].rearrange("p (t k) -> p t k", k=2)
            Erc = ein[:, RCOL:RCOL + 516].rearrange("p (t k) -> p t k", k=2)
            Et = ein[:, TOPS:TOPS + 516].rearrange("p (r m) -> p r m", m=WPAD)
            Eb = ein[:, BOTS:BOTS + 516].rearrange("p (r m) -> p r m", m=WPAD)

            def emit_edges():
                esb = constp.tile([128, EOUT], BF16, name="esb")
                # left (d=5) on (lo,lo)+(up,up); right (d=3) crossed onto
                # (lo,up)+(up,lo) so all four quadrants stream concurrently.
                # Separate psum tiles per conv: chains sharing psum partitions
                # must not share a bank (pending-zero is per partition+bank).
                psL = pp.tile([128, 128], F32, tag="ps1", name="psL")
                psR = pp.tile([128, 128], F32, tag="ps2", name="psR")
                nE = len(LEFT_TAPS)
                for k in range(nE):
                    dyl, dxl = LEFT_TAPS[k]
                    dyr, dxr = RIGHT_TAPS[k]
                    st, sp = (k == 0), (k == nE - 1)
                    nc.tensor.matmul(psL[lo, 0:128],
                                     wte[lo, ts(LEFT_S - NWM + k, 64)],
                                     Elc[lo, 1 + dyl: 129 + dyl, dxl],
                                     start=st, stop=sp, skip_group_check=True)
                    nc.tensor.matmul(psL[up, 0:128],
                                     wte[up, ts(LEFT_S - NWM + k, 64)],
                                     Elc[up, 129 + dyl: 257 + dyl, dxl],
                                     start=st, stop=sp, skip_group_check=True)
                    nc.tensor.matmul(psR[up, 0:128],
                                     wte[lo, ts(RIGHT_S - NWM + k, 64)],
                                     Erc[lo, 1 + dyr: 129 + dyr, dxr + 1],
                                     start=st, stop=sp, skip_group_check=True)
                    nc.tensor.matmul(psR[lo, 0:128],
                                     wte[up, ts(RIGHT_S - NWM + k, 64)],
                                     Erc[up, 129 + dyr: 257 + dyr, dxr + 1],
                                     start=st, stop=sp, skip_group_check=True)
                nc.scalar.activation(esb[:, 0:128], psL[:, 0:128], AF.Identity,
                                     bias=bias_t[:, B_LEFT:B_LEFT + 1])
                nc.scalar.activation(esb[:, 128:256], psR[:, 0:128],
                                     AF.Identity,
                                     bias=bias_t[:, B_RIGHT:B_RIGHT + 1])
                # top row (d=7) on (lo,lo) / bottom row (d=1) on (up,up)
                ptb = pp.tile([128, 256], F32, tag="ps1", name="ptb")
                nT = len(TOP_TAPS)
                for k in range(nT):
                    dyt, dxt = TOP_TAPS[k]
                    dyb, dxb = BOT_TAPS[k]
                    st, sp = (k == 0), (k == nT - 1)
                    nc.tensor.matmul(ptb[lo, 0:256],
                                     wte[lo, ts(TOP_S - NWM + k, 64)],
                                     Et[lo, dyt, dxt + 1: dxt + 257],
                                     start=st, stop=sp, skip_group_check=True)
                    nc.tensor.matmul(ptb[up, 0:256],
                                     wte[up, ts(BOT_S - NWM + k, 64)],
                                     Eb[up, 1 + dyb, dxb + 1: dxb + 257],
                                     start=st, stop=sp, skip_group_check=True)
                nc.scalar.activation(esb[:, 256:512], ptb[:, 0:256],
                                     AF.Identity,
                                     bias=bias_t[:, B_TOPBOT:B_TOPBOT + 1])
                nc.sync.dma_start(edg_d[:], esb[:])

            # ---- dense interior conv (d=4) ----
            blo = bias_t[:, B_MAIN:B_MAIN + 1]
            X0av = X0a[:].rearrange("p (t m) -> p t m", m=WPAD)
            X0bv = X0b[:].rearrange("p (t m) -> p t m", m=WPAD)
            X0cv = X0c[:].rearrange("p (t m) -> p t m", m=WPAD)
            for s in range(NSTRIP):
                if s + 3 < NSTRIP:
                    Xs[s + 3] = xp.tile([128, XCOLS], BF16,
                                        name=f"X{s + 3}", tag="X")
                    nc.sync.dma_start(Xs[s + 3][:], ip[s + 3])
                if s == 0:
                    gviews = [(X0av, 0), (X0bv, -4), (X0bv, -4), (X0cv, -12)]
                else:
                    Xv = Xs.pop(s)[:].rearrange("p (t m) -> p t m", m=WPAD)
                    gviews = [(Xv, 0)] * NG
                last = s == NSTRIP - 1
                if last:
                    # combined tile: group g holds (olo | oup) halves side by
                    # side so each group drains with a single 2-range DMA
                    oL = op.tile([128, H2 * 254], BF16, tag="olo",
                                 name="oL")
                    dstv = out_d[s].rearrange("a p m -> p a m")
                    emit_edges()
                else:
                    olo = op.tile([128, H2 * 254 // 2], BF16, tag="olo")
                    oup = op.tile([128, H2 * 254 // 2], BF16, tag="oup")

                for g in range(NG):
                    Xv, ioff = gviews[g]
                    # bank-sized tiles keep PSUM allocation aligned; only
                    # the first 508 columns are written/read
                    ps1 = pp.tile([128, 512], F32, tag="ps1")
                    ps2 = pp.tile([128, 512], F32, tag="ps2")
                    for k, (dy, dx) in enumerate(TAPS9):
                        st, sp = (k == 0), (k == 8)
                        for (ph, po, i) in ((lo, slice(0, 64), 4 * g),
                                            (up, slice(0, 64), 4 * g),
                                            (lo, slice(64, 128), 4 * g + 2),
                                            (up, slice(64, 128), 4 * g + 2)):
                            psd = (ps1 if ph == lo else ps2)
                            ib = i + ioff
                            rhs = Xv[ph, ib + 1 + dy: ib + 3 + dy,
                                     dx + 2: dx + 256]
                            nc.tensor.matmul(
                                psd[po, 0:508],
                                wtm[ph, ts(k, 64)], rhs,
                                start=st, stop=sp, skip_group_check=True)
                    # evacuate: bias add psum -> sbuf; alternate engines
                    if last:
                        d0 = oL[:, 1016 * g: 1016 * g + 508]
                        d1 = oL[:, 1016 * g + 508: 1016 * (g + 1)]
                    else:
                        d0 = olo[:, ts(g, 508)]
                        d1 = oup[:, ts(g, 508)]
                    if g % 2 == 0:
                        nc.scalar.activation(d0, ps1[:, 0:508], AF.Identity,
                                             bias=blo)
                        nc.vector.tensor_scalar_add(d1, ps2[:, 0:508], blo)
                    else:
                        nc.vector.tensor_scalar_add(d0, ps1[:, 0:508], blo)
                        nc.scalar.activation(d1, ps2[:, 0:508], AF.Identity,
                                             bias=blo)
                    if last:
                        # fine-grained drain so the tail overlaps compute
                        nc.sync.dma_start(
                            dstv[:, :, ts(g, 508)],
                            oL[:, ts(g, 1016)].rearrange("p (a m) -> p a m",
                                                         m=508))
                if not last:
                    nc.sync.dma_start(out_d[s, 0], olo[:])
                    nc.sync.dma_start(out_d[s, 1], oup[:])

    nc.compile()
    return nc


def _get_nc():
    if "nc" not in _CACHE:
        _CACHE["nc"] = _build()
    return _CACHE["nc"]


def _prep_img(imgc):
    """[64,256,256] f32 -> [NSTRIP,128,XCOLS] padded bf16 strip layout."""
    ip = np.zeros((NSTRIP, 2, 64, SLOTS, WPAD), BF)
    for s in range(NSTRIP):
        for g in range(2):
            base = s * R + g * H2 - 1
            l0 = max(0, -base)
            h0 = min(SLOTS, H - base)
            ip[s, g, :, l0:h0, 1:257] = imgc[:, base + l0: base + h0, :]
    return np.ascontiguousarray(ip.reshape(NSTRIP, 128, XCOLS))


def _prep_edge_in(imgc):
    L = np.zeros((64, 258, 2), BF)
    L[:, 1:257, :] = imgc[:, :, 0:2]
    Rt = np.zeros((64, 258, 2), BF)
    Rt[:, 1:257, :] = imgc[:, :, 254:256]
    T = np.zeros((64, 2, WPAD), BF)
    T[:, :, 1:257] = imgc[:, 0:2, :]
    Bo = np.zeros((64, 2, WPAD), BF)
    Bo[:, :, 1:257] = imgc[:, 254:256, :]
    half = np.concatenate([L.reshape(64, 516), Rt.reshape(64, 516),
                           T.reshape(64, 516), Bo.reshape(64, 516)], axis=1)
    return np.ascontiguousarray(np.concatenate([half, half], axis=0))


def _prep_wt(weights):
    wt = np.zeros((128, NW, 64), BF)
    for (d, taps), base in zip(W_GROUPS, _offs):
        for k, (dy, dx) in enumerate(taps):
            m = weights[d][:, :, dy + 1, dx + 1].T  # [cin, cout]
            wt[0:64, base + k] = m
            wt[64:128, base + k] = m
    wt = wt.reshape(128, NW * 64)
    return (np.ascontiguousarray(wt[:, :NWM * 64]),
            np.ascontiguousarray(wt[:, NWM * 64:]))


def _prep_bias(bias):
    bs = np.zeros((128, NB), np.float32)
    for c, (dl, du) in enumerate(BIAS_PAIRS):
        bs[0:64, c] = bias[dl]
        bs[64:128, c] = bias[du]
    return bs


def _make_in_maps(img, weights, bias):
    img = np.asarray(img, np.float32)
    wtm, wte = _prep_wt(np.asarray(weights, np.float32))
    bs = _prep_bias(np.asarray(bias, np.float32))
    return [{"img_prep": _prep_img(img[c]), "edge_in": _prep_edge_in(img[c]),
             "wtm": wtm, "wte": wte, "bias": bs}
            for c in range(NCORES)]


def _unprep_out(o, e):
    """Assemble [C,H,W] from dense out + edge overlay (corners excluded)."""
    e = e.astype(np.float32)
    v = o.astype(np.float32).reshape(NSTRIP, 2, 2, 64, 4, 2, 254)
    out = np.empty((C, H, W), np.float32)
    out[:, :, 1:255] = v.transpose(3, 0, 1, 4, 2, 5, 6).reshape(C, H, 254)
    Lv = np.concatenate([e[0:64, 0:128], e[64:128, 0:128]], axis=1)
    # right chain is crossed over psum partition halves
    Rv = np.concatenate([e[64:128, 128:256], e[0:64, 128:256]], axis=1)
    out[:, 1:255, 0] = Lv[:, 1:255]
    out[:, 1:255, 255] = Rv[:, 1:255]
    out[:, 0, 1:255] = e[0:64, 257:511]
    out[:, 255, 1:255] = e[64:128, 257:511]
    return out


def _fix_corners(out, imgc, weights, bias):
    """The 4 corner pixels (convs d in {8,6,2,0}) computed host-side."""
    out[:, 0, 0] = np.einsum('oikl,ikl->o', weights[8][:, :, 1:3, 1:3],
                             imgc[:, 0:2, 0:2]) + bias[8]
    out[:, 0, 255] = np.einsum('oikl,ikl->o', weights[6][:, :, 1:3, 0:2],
                               imgc[:, 0:2, 254:256]) + bias[6]
    out[:, 255, 0] = np.einsum('oikl,ikl->o', weights[2][:, :, 0:2, 1:3],
                               imgc[:, 254:256, 0:2]) + bias[2]
    out[:, 255, 255] = np.einsum('oikl,ikl->o', weights[0][:, :, 0:2, 0:2],
                                 imgc[:, 254:256, 254:256]) + bias[0]


def _assemble(res, img, weights, bias):
    img = np.asarray(img, np.float32)
    weights = np.asarray(weights, np.float32)
    bias = np.asarray(bias, np.float32)
    outs = []
    for c in range(NCORES):
        out = _unprep_out(res.results[c]["out"], res.results[c]["edges"])
        _fix_corners(out, img[c], weights, bias)
        outs.append(out)
    return np.stack(outs)


def kernel(img, weights, bias):
    nc = _get_nc()
    in_maps = _make_in_maps(img, weights, bias)
    res = run_bass_kernel_spmd(nc, in_maps, list(range(NCORES)))
    return _assemble(res, img, weights, bias)


# revision 37
# speedup vs baseline: 1.0112x; 1.0112x over previous
"""bf16 variant: 4-way PE-quadrant packing + once-per-image edge compute.

Main conv (d=4): four concurrent K=64/M=64 bf16 matmuls occupy the four 64x64
quadrants of the PE array (tile positions (0,0), (0,64), (64,0), (64,64)):
lower/upper image half on array rows, even/odd row-pair on array columns.
Each group of 4 row-pairs runs 9 taps x 4 units; evacuation (bias add,
psum->sbuf) alternates between ScalarE and VectorE.

Timeline engineering:
- PE warmup: ~16 dummy taps on a memset scratch tile run during the initial
  DMA window so the TensorE p-state is at max clock when real data lands.
- Startup-critical DMA order: main-tap weights (148KB), then strip 0 as two
  independent tiles (slots 0-9 / 8-17, 2-slot halo duplicated) so groups 0-1
  start as soon as the first half lands. Edge tensors stream behind strips
  1-2.
- Edge frame (convs d in {7,1,5,3}) is computed once per image AFTER the
  dense loop (overlapping the output-DMA drain), with left/right chains
  crossed over psum partition halves so all four PE quadrants run. The four
  corner pixels are fixed up on the host. Host overlays edges onto the dense
  result. Dense + edge outputs are bf16 (error budget allows it).
"""

import ml_dtypes
import numpy as np

import concourse.bacc as bacc
import concourse.mybir as mybir
import concourse.tile as tile
from concourse.bass import ts
from concourse.bass_utils import run_bass_kernel_spmd

B, C, H, W = 8, 64, 256, 256
NCORES = 8
R = 32              # output rows per strip
H2 = R // 2         # rows per partition-half
NSTRIP = H // R
SLOTS = H2 + 2
WPAD = W + 2
XCOLS = SLOTS * WPAD
NG = H2 // 4        # groups of 4 row-pairs per half
F32 = mybir.dt.float32
BF16 = mybir.dt.bfloat16
AF = mybir.ActivationFunctionType
BF = ml_dtypes.bfloat16
NWARM = 20          # PE p-state warmup taps (256 rows, ~0.21us each at mid
                    # clock); must END at/after first-input-ready: an idle
                    # Tensor gap before real work resets the p-state ramp

TAPS9 = [(dy, dx) for dy in (-1, 0, 1) for dx in (-1, 0, 1)]
TOP_TAPS = [(dy, dx) for dy in (0, 1) for dx in (-1, 0, 1)]      # d=7 row 0
BOT_TAPS = [(dy, dx) for dy in (-1, 0) for dx in (-1, 0, 1)]     # d=1 row 255
LEFT_TAPS = [(dy, dx) for dy in (-1, 0, 1) for dx in (0, 1)]     # d=5 col 0
RIGHT_TAPS = [(dy, dx) for dy in (-1, 0, 1) for dx in (-1, 0)]   # d=3 col 255

W_GROUPS = [
    (4, TAPS9), (7, TOP_TAPS), (1, BOT_TAPS), (5, LEFT_TAPS), (3, RIGHT_TAPS),
]
_offs = []
_acc = 0
for _d, _taps in W_GROUPS:
    _offs.append(_acc)
    _acc += len(_taps)
(MAIN_S, TOP_S, BOT_S, LEFT_S, RIGHT_S) = _offs
NW = _acc  # 33
NWM = len(TAPS9)       # main-tap weight columns (first group)
NWE = NW - NWM         # edge-tap weight columns
# bias tile [128, NB]: column -> (value on partitions 0-63, on 64-127)
BIAS_PAIRS = [(4, 4), (5, 5), (3, 3), (7, 1)]
B_MAIN, B_LEFT, B_RIGHT, B_TOPBOT = range(4)
NB = 4

# edge_in sections (element offsets per partition)
LCOL, RCOL, TOPS, BOTS = 0, 516, 1032, 1548
EIN = 2064
EOUT = 512  # edges_out: [0:128] left, [128:256] right (row-half crossed),
            # [256:512] top|bottom

_CACHE = {}


def _build():
    nc = bacc.Bacc("TRN2", target_bir_lowering=False, debug=False,
                   num_devices=NCORES)
    ip = nc.dram_tensor("img_prep", [NSTRIP, 128, XCOLS], BF16,
                        kind="ExternalInput").ap()
    ein_d = nc.dram_tensor("edge_in", [128, EIN], BF16,
                           kind="ExternalInput").ap()
    wtm_d = nc.dram_tensor("wtm", [128, NWM * 64], BF16,
                           kind="ExternalInput").ap()
    wte_d = nc.dram_tensor("wte", [128, NWE * 64], BF16,
                           kind="ExternalInput").ap()
    bias_d = nc.dram_tensor("bias", [128, NB], F32, kind="ExternalInput").ap()
    out_d = nc.dram_tensor("out", [NSTRIP, 2, 128, H2 * 254 // 2], BF16,
                           kind="ExternalOutput").ap()
    edg_d = nc.dram_tensor("edges", [128, EOUT], BF16,
                           kind="ExternalOutput").ap()
    warm_d = nc.dram_tensor("warm", [128, 4], F32, kind="ExternalOutput").ap()

    lo, up = slice(0, 64), slice(64, 128)
    quads = ((lo, slice(0, 64)), (up, slice(0, 64)),
             (lo, slice(64, 128)), (up, slice(64, 128)))

    with tile.TileContext(nc) as tc:
        with (
            tc.tile_pool(name="const", bufs=1) as constp,
            tc.tile_pool(name="xin", bufs=5) as xp,
            tc.tile_pool(name="outp", bufs=5) as op,
            tc.tile_pool(name="psmain", bufs=4, space="PSUM") as pp,
        ):
            # ---- PE p-state warmup on a memset scratch (no DMA deps) ----
            if NWARM:
                warm = constp.tile([128, 576], BF16)
                nc.gpsimd.memset(warm[:], 0.0)
                wps1 = pp.tile([128, 512], F32, tag="ps1")
                wps2 = pp.tile([128, 512], F32, tag="ps2")
                for k in range(NWARM):
                    st, sp = (k == 0), (k == NWARM - 1)
                    for (ph, po) in quads:
                        psd = (wps1 if ph == lo else wps2)
                        nc.tensor.matmul(psd[po, 0:256], warm[ph, 0:64],
                                         warm[ph, 64:320], start=st, stop=sp,
                                         skip_group_check=True)

            # ---- startup-critical DMAs first on the Sync hardware ring
            # (other engines' dma_start lands on the slow Q0 software path).
            # Strip 0 comes as 3 tiles (6/10/6 slots, 2-slot halos duplicated)
            # so group 0 can start on a 0.4MB chunk while the ring ramps.
            wtm = constp.tile([128, NWM * 64], BF16)
            nc.sync.dma_start(wtm[:], wtm_d[:])
            X0a = constp.tile([128, 6 * WPAD], BF16)
            nc.sync.dma_start(X0a[:], ip[0][:, 0:6 * WPAD])
            X0b = constp.tile([128, 10 * WPAD], BF16)
            nc.sync.dma_start(X0b[:], ip[0][:, 4 * WPAD:14 * WPAD])
            X0c = constp.tile([128, 6 * WPAD], BF16)
            nc.sync.dma_start(X0c[:], ip[0][:, 12 * WPAD:18 * WPAD])
            bias_t = constp.tile([128, NB], F32)
            nc.sync.dma_start(bias_t[:], bias_d[:])
            Xs = {}
            for s in (1, 2):
                Xs[s] = xp.tile([128, XCOLS], BF16, name=f"X{s}", tag="X")
                nc.sync.dma_start(Xs[s][:], ip[s])
            # edge tensors + warmup readback stream behind the first strips
            ein = constp.tile([128, EIN], BF16)
            nc.sync.dma_start(ein[:], ein_d[:])
            wte = constp.tile([128, NWE * 64], BF16)
            nc.sync.dma_start(wte[:], wte_d[:])
            if NWARM:
                # readback gives the warmup psum tiles readers; emitted after
                # the descriptor issues so it never gates them
                wsb = constp.tile([128, 4], F32)
                nc.scalar.activation(wsb[:, 0:2], wps1[:, 0:2], AF.Identity)
                nc.scalar.activation(wsb[:, 2:4], wps2[:, 0:2], AF.Identity)
                nc.sync.dma_start(warm_d[:], wsb[:])

            # ---- edge frame emitter (called between strips 6 and 7 so the
            # edge evac/DMA overlap the last strip's compute) ----
            Elc = ein[:, LCOL:LCOL + 516].rearrange("p (t k) -> p t k", k=2)
            Erc = ein[:, RCOL:RCOL + 516].rearrange("p (t k) -> p t k", k=2)
            Et = ein[:, TOPS:TOPS + 516].rearrange("p (r m) -> p r m", m=WPAD)
            Eb = ein[:, BOTS:BOTS + 516].rearrange("p (r m) -> p r m", m=WPAD)

            def emit_edges():
                esb = constp.tile([128, EOUT], BF16, name="esb")
                # left (d=5) on (lo,lo)+(up,up); right (d=3) crossed onto
                # (lo,up)+(up,lo) so all four quadrants stream concurrently.
                # Separate psum tiles per conv: chains sharing psum partitions
                # must not share a bank (pending-zero is per partition+bank).
                psL = pp.tile([128, 128], F32, tag="ps1", name="psL")
                psR = pp.tile([128, 128], F32, tag="ps2", name="psR")
                nE = len(LEFT_TAPS)
                for k in range(nE):
                    dyl, dxl = LEFT_TAPS[k]
                    dyr, dxr = RIGHT_TAPS[k]
                    st, sp = (k == 0), (k == nE - 1)
                    nc.tensor.matmul(psL[lo, 0:128],
                                     wte[lo, ts(LEFT_S - NWM + k, 64)],
                                     Elc[lo, 1 + dyl: 129 + dyl, dxl],
                                     start=st, stop=sp, skip_group_check=True)
                    nc.tensor.matmul(psL[up, 0:128],
                                     wte[up, ts(LEFT_S - NWM + k, 64)],
                                     Elc[up, 129 + dyl: 257 + dyl, dxl],
                                     start=st, stop=sp, skip_group_check=True)
                    nc.tensor.matmul(psR[up, 0:128],
                                     wte[lo, ts(RIGHT_S - NWM + k, 64)],
                                     Erc[lo, 1 + dyr: 129 + dyr, dxr + 1],
                                     start=st, stop=sp, skip_group_check=True)
                    nc.tensor.matmul(psR[lo, 0:128],
                                     wte[up, ts(RIGHT_S - NWM + k, 64)],
                                     Erc[up, 129 + dyr: 257 + dyr, dxr + 1],
                                     start=st, stop=sp, skip_group_check=True)
                nc.scalar.activation(esb[:, 0:128], psL[:, 0:128], AF.Identity,
                                     bias=bias_t[:, B_LEFT:B_LEFT + 1])
                nc.scalar.activation(esb[:, 128:256], psR[:, 0:128],
                                     AF.Identity,
                                     bias=bias_t[:, B_RIGHT:B_RIGHT + 1])
                # top row (d=7) on (lo,lo) / bottom row (d=1) on (up,up)
                ptb = pp.tile([128, 256], F32, tag="ps1", name="ptb")
                nT = len(TOP_TAPS)
                for k in range(nT):
                    dyt, dxt = TOP_TAPS[k]
                    dyb, dxb = BOT_TAPS[k]
                    st, sp = (k == 0), (k == nT - 1)
                    nc.tensor.matmul(ptb[lo, 0:256],
                                     wte[lo, ts(TOP_S - NWM + k, 64)],
                                     Et[lo, dyt, dxt + 1: dxt + 257],
                                     start=st, stop=sp, skip_group_check=True)
                    nc.tensor.matmul(ptb[up, 0:256],
                                     wte[up, ts(BOT_S - NWM + k, 64)],
                                     Eb[up, 1 + dyb, dxb + 1: dxb + 257],
                                     start=st, stop=sp, skip_group_check=True)
                nc.scalar.activation(esb[:, 256:512], ptb[:, 0:256],
                                     AF.Identity,
                                     bias=bias_t[:, B_TOPBOT:B_TOPBOT + 1])
                nc.sync.dma_start(edg_d[:], esb[:])

            # ---- dense interior conv (d=4) ----
            blo = bias_t[:, B_MAIN:B_MAIN + 1]
            X0av = X0a[:].rearrange("p (t m) -> p t m", m=WPAD)
            X0bv = X0b[:].rearrange("p (t m) -> p t m", m=WPAD)
            X0cv = X0c[:].rearrange("p (t m) -> p t m", m=WPAD)
            for s in range(NSTRIP):
                if s + 3 < NSTRIP:
                    Xs[s + 3] = xp.tile([128, XCOLS], BF16,
                                        name=f"X{s + 3}", tag="X")
                    nc.sync.dma_start(Xs[s + 3][:], ip[s + 3])
                if s == 0:
                    gviews = [(X0av, 0), (X0bv, -4), (X0bv, -4), (X0cv, -12)]
                else:
                    Xv = Xs.pop(s)[:].rearrange("p (t m) -> p t m", m=WPAD)
                    gviews = [(Xv, 0)] * NG
                last = s == NSTRIP - 1
                if last:
                    # combined tile: group g holds (olo | oup) halves side by
                    # side so each group drains with a single 2-range DMA
                    oL = op.tile([128, H2 * 254], BF16, tag="olo",
                                 name="oL")
                    dstv = out_d[s].rearrange("a p m -> p a m")
                    emit_edges()
                else:
                    olo = op.tile([128, H2 * 254 // 2], BF16, tag="olo")
                    oup = op.tile([128, H2 * 254 // 2], BF16, tag="oup")

                for g in range(NG):
                    Xv, ioff = gviews[g]
                    # bank-sized tiles keep PSUM allocation aligned; only
                    # the first 508 columns are written/read
                    ps1 = pp.tile([128, 512], F32, tag="ps1")
                    ps2 = pp.tile([128, 512], F32, tag="ps2")
                    for k, (dy, dx) in enumerate(TAPS9):
                        st, sp = (k == 0), (k == 8)
                        for (ph, po, i) in ((lo, slice(0, 64), 4 * g),
                                            (up, slice(0, 64), 4 * g),
                                            (lo, slice(64, 128), 4 * g + 2),
                                            (up, slice(64, 128), 4 * g + 2)):
                            psd = (ps1 if ph == lo else ps2)
                            ib = i + ioff
                            rhs = Xv[ph, ib + 1 + dy: ib + 3 + dy,
                                     dx + 2: dx + 256]
                            nc.tensor.matmul(
                                psd[po, 0:508],
                                wtm[ph, ts(k, 64)], rhs,
                                start=st, stop=sp, skip_group_check=True)
                    # evacuate: bias add psum -> sbuf; alternate engines
                    if last:
                        d0 = oL[:, 1016 * g: 1016 * g + 508]
                        d1 = oL[:, 1016 * g + 508: 1016 * (g + 1)]
                    else:
                        d0 = olo[:, ts(g, 508)]
                        d1 = oup[:, ts(g, 508)]
                    if g % 2 == 0:
                        nc.scalar.activation(d0, ps1[:, 0:508], AF.Identity,
                                             bias=blo)
                        nc.vector.tensor_scalar_add(d1, ps2[:, 0:508], blo)
                    else:
                        nc.vector.tensor_scalar_add(d0, ps1[:, 0:508], blo)
                        nc.scalar.activation(d1, ps2[:, 0:508], AF.Identity,
                                             bias=blo)
                    if last:
                        # fine-grained drain so the tail overlaps compute
                        nc.sync.dma_start(
                            dstv[:, :, ts(g, 508)],
                            oL[:, ts(g, 1016)].rearrange("p (a m) -> p a m",
                                                         m=508))
                if not last:
                    nc.sync.dma_start(out_d[s, 0], olo[:])
                    nc.sync.dma_start(out_d[s, 1], oup[:])

    nc.compile()
    return nc


def _get_nc():
    if "nc" not in _CACHE:
        _CACHE["nc"] = _build()
    return _CACHE["nc"]


def _prep_img(imgc):
    """[64,256,256] f32 -> [NSTRIP,128,XCOLS] padded bf16 strip layout."""
    ip = np.zeros((NSTRIP, 2, 64, SLOTS, WPAD), BF)
    for s in range(NSTRIP):
        for g in range(2):
            base = s * R + g * H2 - 1
            l0 = max(0, -base)
            h0 = min(SLOTS, H - base)
            ip[s, g, :, l0:h0, 1:257] = imgc[:, base + l0: base + h0, :]
    return np.ascontiguousarray(ip.reshape(NSTRIP, 128, XCOLS))


def _prep_edge_in(imgc):
    L = np.zeros((64, 258, 2), BF)
    L[:, 1:257, :] = imgc[:, :, 0:2]
    Rt = np.zeros((64, 258, 2), BF)
    Rt[:, 1:257, :] = imgc[:, :, 254:256]
    T = np.zeros((64, 2, WPAD), BF)
    T[:, :, 1:257] = imgc[:, 0:2, :]
    Bo = np.zeros((64, 2, WPAD), BF)
    Bo[:, :, 1:257] = imgc[:, 254:256, :]
    half = np.concatenate([L.reshape(64, 516), Rt.reshape(64, 516),
                           T.reshape(64, 516), Bo.reshape(64, 516)], axis=1)
    return np.ascontiguousarray(np.concatenate([half, half], axis=0))


def _prep_wt(weights):
    wt = np.zeros((128, NW, 64), BF)
    for (d, taps), base in zip(W_GROUPS, _offs):
        for k, (dy, dx) in enumerate(taps):
            m = weights[d][:, :, dy + 1, dx + 1].T  # [cin, cout]
            wt[0:64, base + k] = m
            wt[64:128, base + k] = m
    wt = wt.reshape(128, NW * 64)
    return (np.ascontiguousarray(wt[:, :NWM * 64]),
            np.ascontiguousarray(wt[:, NWM * 64:]))


def _prep_bias(bias):
    bs = np.zeros((128, NB), np.float32)
    for c, (dl, du) in enumerate(BIAS_PAIRS):
        bs[0:64, c] = bias[dl]
        bs[64:128, c] = bias[du]
    return bs


def _make_in_maps(img, weights, bias):
    img = np.asarray(img, np.float32)
    wtm, wte = _prep_wt(np.asarray(weights, np.float32))
    bs = _prep_bias(np.asarray(bias, np.float32))
    return [{"img_prep": _prep_img(img[c]), "edge_in": _prep_edge_in(img[c]),
             "wtm": wtm, "wte": wte, "bias": bs}
            for c in range(NCORES)]


def _unprep_out(o, e):
    """Assemble [C,H,W] from dense out + edge overlay (corners excluded)."""
    e = e.astype(np.float32)
    v = o.astype(np.float32).reshape(NSTRIP, 2, 2, 64, 4, 2, 254)
    out = np.empty((C, H, W), np.float32)
    out[:, :, 1:255] = v.transpose(3, 0, 1, 4, 2, 5, 6).reshape(C, H, 254)
    Lv = np.concatenate([e[0:64, 0:128], e[64:128, 0:128]], axis=1)
    # right chain is crossed over psum partition halves
    Rv = np.concatenate([e[64:128, 128:256], e[0:64, 128:256]], axis=1)
    out[:, 1:255, 0] = Lv[:, 1:255]
    out[:, 1:255, 255] = Rv[:, 1:255]
    out[:, 0, 1:255] = e[0:64, 257:511]
    out[:, 255, 1:255] = e[64:128, 257:511]
    return out


def _fix_corners(out, imgc, weights, bias):
    """The 4 corner pixels (convs d in {8,6,2,0}) computed host-side."""
    out[:, 0, 0] = np.einsum('oikl,ikl->o', weights[8][:, :, 1:3, 1:3],
                             imgc[:, 0:2, 0:2]) + bias[8]
    out[:, 0, 255] = np.einsum('oikl,ikl->o', weights[6][:, :, 1:3, 0:2],
                               imgc[:, 0:2, 254:256]) + bias[6]
    out[:, 255, 0] = np.einsum('oikl,ikl->o', weights[2][:, :, 0:2, 1:3],
                               imgc[:, 254:256, 0:2]) + bias[2]
    out[:, 255, 255] = np.einsum('oikl,ikl->o', weights[0][:, :, 0:2, 0:2],
                                 imgc[:, 254:256, 254:256]) + bias[0]


def _assemble(res, img, weights, bias):
    img = np.asarray(img, np.float32)
    weights = np.asarray(weights, np.float32)
    bias = np.asarray(bias, np.float32)
    outs = []
    for c in range(NCORES):
        out = _unprep_out(res.results[c]["out"], res.results[c]["edges"])
        _fix_corners(out, img[c], weights, bias)
        outs.append(out)
    return np.stack(outs)


def kernel(img, weights, bias):
    nc = _get_nc()
    in_maps = _make_in_maps(img, weights, bias)
    res = run_bass_kernel_spmd(nc, in_maps, list(range(NCORES)))
    return _assemble(res, img, weights, bias)
